# revision 9
# baseline (speedup 1.0000x reference)
"""Multi-head self-attention with RoPE — Trainium2 Bass/Tile kernel, 8 NeuronCores.

Sharding: batch x head tensor-parallel. Core pair (2b, 2b+1) handles batch b;
within a pair each core computes 8 of the 16 heads (W_q/W_k/W_v column-sharded,
W_o row-sharded), then a pairwise ReduceScatter sums the output-projection
partials and leaves each core with half of its batch's sequence rows.

Device layout notes:
 - All projections contract d_model on the partition dim; Q/K are produced
   transposed [d_k, seq] per head so attention scores come out transposed
   [k, q] ("S^T" layout): softmax reduction runs across partitions (GpSimd
   partition_all_reduce) and the AV matmul needs no transposes at all.
 - RoPE is applied via a host-side even/odd permutation of the W_q/W_k rows
   plus [cos;cos] and [sin;-sin] tables; the partition-half swap is done with
   two SBUF->SBUF DMAs.
 - No max-subtraction in softmax: scores here are bounded (|s| < ~10), exp is
   safe in f32/bf16. Causal masking adds -60 to masked diagonal-block entries
   before exp.
 - Matmuls run in bf16 with f32 PSUM accumulation; 1/sqrt(d_k) is folded into
   W_q on the host.
"""
import numpy as np
import ml_dtypes

D_MODEL = 2048
N_HEADS = 16
D_K = 128
B = 4
S = 2048
THETA = 10000.0
N_CORES = 8
HPC = N_HEADS // 2     # heads per core
HROWS = HPC * D_K      # 1024 = per-core projection width
NQT = S // 512         # 4 q-tiles of 512
NKC = S // 128         # 16 k-chunks of 128
NEG = -60.0
BF16 = ml_dtypes.bfloat16

_cache = {}


def _host_prep(x, token_positions, W_q, W_k, W_v, W_o):
    """Per-core input maps (sharding + layout prep, all host-side numpy)."""
    x = np.asarray(x, np.float32)
    W_q = np.asarray(W_q, np.float32)
    W_k = np.asarray(W_k, np.float32)
    W_v = np.asarray(W_v, np.float32)
    W_o = np.asarray(W_o, np.float32)
    pos = np.asarray(token_positions).astype(np.float32)

    half = D_K // 2
    inv_freq = (THETA ** (-(np.arange(half, dtype=np.float32) * 2.0) / D_K)).astype(np.float32)
    ang = pos[:, None] * inv_freq[None, :]          # [S, 64]
    cos = np.cos(ang).astype(np.float32).T          # [64, S]
    sin = np.sin(ang).astype(np.float32).T
    cos2 = np.concatenate([cos, cos], axis=0)                # [128, S] f32
    sin2 = np.concatenate([sin, -sin], axis=0)               # [128, S] f32

    perm = np.concatenate([np.arange(0, D_K, 2), np.arange(1, D_K, 2)])

    kl = np.arange(128)[:, None, None]
    dd = np.arange(4)[None, :, None]
    jj = np.arange(512)[None, None, :]
    masks = np.where(dd * 128 + kl <= jj, 0.0, NEG).astype(np.float32)  # [128,4,512]

    in_maps = []
    for c in range(N_CORES):
        b = c // 2
        hh = c % 2
        hsel = slice(hh * HROWS, (hh + 1) * HROWS)

        def permute_heads(Wrows):
            Wr = Wrows.reshape(HPC, D_K, D_MODEL)[:, perm, :]
            return Wr.reshape(HROWS, D_MODEL)

        wq = permute_heads(W_q[hsel]) / np.sqrt(np.float32(D_K))
        wk = permute_heads(W_k[hsel])
        wv = W_v[hsel]
        wo = W_o[:, hsel]                            # [2048, 1024]

        in_maps.append({
            "xT": np.ascontiguousarray(x[b].T).astype(BF16),     # [2048, 2048]
            "wqT": np.ascontiguousarray(wq.T).astype(BF16),      # [2048, 1024]
            "wkT": np.ascontiguousarray(wk.T).astype(BF16),      # [2048, 1024]
            "wvT": np.ascontiguousarray(wv.T).astype(BF16),      # [2048, 1024]
            "woT": np.ascontiguousarray(wo.T).astype(BF16),      # [1024, 2048]
            "cos2": cos2,
            "sin2": sin2,
            "masks": masks,
        })
    return in_maps


def _build_program(use_collective=True):
    import concourse.bass as bass
    import concourse.mybir as mybir
    import concourse.tile as tile
    from concourse import bacc, bass_isa

    f32 = mybir.dt.float32
    bf16 = mybir.dt.bfloat16
    EXP = mybir.ActivationFunctionType.Exp
    MUL = mybir.AluOpType.mult
    ADD = mybir.AluOpType.add

    nc = bacc.Bacc("TRN2", target_bir_lowering=False, debug=False,
                   num_devices=N_CORES)

    xT_d = nc.dram_tensor("xT", [D_MODEL, S], bf16, kind="ExternalInput")
    wqT_d = nc.dram_tensor("wqT", [D_MODEL, HROWS], bf16, kind="ExternalInput")
    wkT_d = nc.dram_tensor("wkT", [D_MODEL, HROWS], bf16, kind="ExternalInput")
    wvT_d = nc.dram_tensor("wvT", [D_MODEL, HROWS], bf16, kind="ExternalInput")
    woT_d = nc.dram_tensor("woT", [HROWS, D_MODEL], bf16, kind="ExternalInput")
    cos2_d = nc.dram_tensor("cos2", [128, S], f32, kind="ExternalInput")
    sin2_d = nc.dram_tensor("sin2", [128, S], f32, kind="ExternalInput")
    masks_d = nc.dram_tensor("masks", [128, 4, 512], f32, kind="ExternalInput")
    out_d = nc.dram_tensor("out", [S // 2 if use_collective else S, D_MODEL], f32,
                           kind="ExternalOutput")

    DM_CH = D_MODEL // 128  # 16 contraction chunks

    with tile.TileContext(nc) as tc:
        with (
            tc.tile_pool(name="const", bufs=1) as cpool,
            tc.tile_pool(name="big", bufs=1) as bigpool,
            tc.tile_pool(name="xs", bufs=1) as xpool,
            tc.tile_pool(name="w", bufs=2) as wpool,
            tc.tile_pool(name="qt", bufs=2) as qpool,
            tc.tile_pool(name="tmp", bufs=2) as tpool,
            tc.tile_pool(name="den", bufs=1) as dpool,
            tc.tile_pool(name="p", bufs=2) as ppool,
            tc.tile_pool(name="osb", bufs=2) as opool,
            tc.tile_pool(name="psum", bufs=2, space="PSUM") as psum,
            tc.tile_pool(name="dram", bufs=1, space="DRAM") as dram,
        ):
            # ---- constants ----
            cos2 = cpool.tile([128, S], f32, tag="cos2")
            sin2 = cpool.tile([128, S], f32, tag="sin2")
            masks = cpool.tile([128, 4, 512], f32, tag="masks")
            nc.sync.dma_start(cos2[:], cos2_d[:])
            nc.sync.dma_start(sin2[:], sin2_d[:])
            nc.sync.dma_start(masks[:], masks_d[:])

            # ---- persistent phase-A outputs ----
            kTr = bigpool.tile([128, HPC, S], bf16, tag="kTr")      # [dk, h, keys]
            v_sb = bigpool.tile([128, NKC, HROWS], bf16, tag="v")   # [row, kc, hdim]

            # DRAM bounce buffers for the collective
            pout = dram.tile([S, D_MODEL], f32, tag="pout")
            rs_out = dram.tile([S // 2, D_MODEL], f32, tag="rs_out")

            def rope_epilogue(ps, out_ap, ns):
                """out = ps*cos2 + swap(ps*sin2), cast bf16. ps: [128,512] psum."""
                u = tpool.tile([128, 512], f32, tag="u")
                t = tpool.tile([128, 512], f32, tag="t")
                usw = tpool.tile([128, 512], f32, tag="usw")
                nc.vector.tensor_tensor(u[:], ps[:], sin2[:, ns], MUL)
                nc.vector.tensor_tensor(t[:], ps[:], cos2[:, ns], MUL)
                nc.sync.dma_start(usw[0:64, :], u[64:128, :])
                nc.sync.dma_start(usw[64:128, :], u[0:64, :])
                nc.vector.tensor_tensor(out_ap, t[:], usw[:], ADD)

            xT_r = xT_d[:].rearrange("(c p) s -> p c s", p=128)     # [128, 16, S]

            # ---- K projection (transposed, RoPE) ----
            for n in range(4):
                ns = slice(n * 512, (n + 1) * 512)
                xs = xpool.tile([128, DM_CH, 512], bf16, tag="xs")
                nc.sync.dma_start(xs[:], xT_r[:, :, ns])
                for m in range(HPC):
                    wt = wpool.tile([128, DM_CH, 128], bf16, tag="w")
                    nc.sync.dma_start(
                        wt[:],
                        wkT_d[:].rearrange("(c p) m -> p c m", p=128)[
                            :, :, m * 128:(m + 1) * 128],
                    )
                    ps = psum.tile([128, 512], f32, tag="proj")
                    for k in range(DM_CH):
                        nc.tensor.matmul(ps[:], wt[:, k, :], xs[:, k, :],
                                         start=(k == 0), stop=(k == DM_CH - 1))
                    rope_epilogue(ps, kTr[:, m, ns], ns)

            # ---- V projection (natural layout) ----
            for n in range(4):
                ns = slice(n * 512, (n + 1) * 512)
                xs = xpool.tile([128, DM_CH, 512], bf16, tag="xs")
                nc.sync.dma_start(xs[:], xT_r[:, :, ns])
                for nv in range(2):
                    nvs = slice(nv * 512, (nv + 1) * 512)
                    wv = wpool.tile([128, DM_CH, 512], bf16, tag="w")
                    nc.sync.dma_start(
                        wv[:],
                        wvT_d[:].rearrange("(c p) m -> p c m", p=128)[:, :, nvs],
                    )
                    for rc in range(4):
                        ps = psum.tile([128, 512], f32, tag="proj")
                        for k in range(DM_CH):
                            nc.tensor.matmul(
                                ps[:], xs[:, k, rc * 128:(rc + 1) * 128],
                                wv[:, k, :],
                                start=(k == 0), stop=(k == DM_CH - 1))
                        nc.scalar.copy(v_sb[:, n * 4 + rc, nvs], ps[:])

            # ---- per q-tile: Q projection + attention + O projection ----
            for qt in range(NQT):
                qs = slice(qt * 512, (qt + 1) * 512)
                xs = xpool.tile([128, DM_CH, 512], bf16, tag="xs")
                nc.sync.dma_start(xs[:], xT_r[:, :, qs])
                qTr = qpool.tile([128, HPC, 512], bf16, tag="qTr")
                for m in range(HPC):
                    wt = wpool.tile([128, DM_CH, 128], bf16, tag="w")
                    nc.sync.dma_start(
                        wt[:],
                        wqT_d[:].rearrange("(c p) m -> p c m", p=128)[
                            :, :, m * 128:(m + 1) * 128],
                    )
                    ps = psum.tile([128, 512], f32, tag="proj")
                    for k in range(DM_CH):
                        nc.tensor.matmul(ps[:], wt[:, k, :], xs[:, k, :],
                                         start=(k == 0), stop=(k == DM_CH - 1))
                    rope_epilogue(ps, qTr[:, m, :], qs)

                ctx_t = qpool.tile([128, HPC, 512], bf16, tag="ctx")
                nkc = 4 * (qt + 1)
                for h in range(HPC):
                    ctx_ps = psum.tile([128, 512], f32, tag="ctx")
                    den = dpool.tile([128, 512], f32, tag="den")
                    for kc in range(nkc):
                        s_ps = psum.tile([128, 512], f32, tag="S")
                        nc.tensor.matmul(
                            s_ps[:], kTr[:, h, kc * 128:(kc + 1) * 128],
                            qTr[:, h, :], start=True, stop=True)
                        d = kc - 4 * qt
                        if d >= 0:
                            nc.vector.tensor_tensor(s_ps[:], s_ps[:],
                                                    masks[:, d, :], ADD)
                        p_sb = ppool.tile([128, 512], bf16, tag="p")
                        nc.scalar.activation(p_sb[:], s_ps[:], EXP)
                        if kc == 0:
                            nc.gpsimd.tensor_copy(den[:], p_sb[:])
                        else:
                            dtmp = dpool.tile([128, 512], f32, tag="dtmp")
                            nc.gpsimd.tensor_copy(dtmp[:], p_sb[:])
                            nc.gpsimd.tensor_tensor(den[:], den[:], dtmp[:], ADD)
                        nc.tensor.matmul(
                            ctx_ps[:], v_sb[:, kc, h * 128:(h + 1) * 128],
                            p_sb[:], start=(kc == 0), stop=(kc == nkc - 1))
                    denr = dpool.tile([128, 512], f32, tag="denr")
                    nc.gpsimd.partition_all_reduce(denr[:], den[:], channels=128,
                                                   reduce_op=bass_isa.ReduceOp.add)
                    nc.vector.reciprocal(denr[:], denr[:])
                    nc.vector.tensor_tensor(ctx_t[:, h, :], ctx_ps[:], denr[:], MUL)

                # O projection for this q-tile's rows
                for nt in range(4):
                    nts = slice(nt * 512, (nt + 1) * 512)
                    wo = wpool.tile([128, HPC, 512], bf16, tag="w")
                    nc.sync.dma_start(
                        wo[:],
                        woT_d[:].rearrange("(c p) m -> p c m", p=128)[:, :, nts],
                    )
                    for rc in range(4):
                        o_ps = psum.tile([128, 512], f32, tag="O")
                        for h in range(HPC):
                            nc.tensor.matmul(
                                o_ps[:], ctx_t[:, h, rc * 128:(rc + 1) * 128],
                                wo[:, h, :], start=(h == 0), stop=(h == HPC - 1))
                        osb = opool.tile([128, 512], f32, tag="osb")
                        nc.scalar.copy(osb[:], o_ps[:])
                        r0 = qt * 512 + rc * 128
                        nc.sync.dma_start(pout[r0:r0 + 128, nts], osb[:])

            # ---- pairwise reduce-scatter + output ----
            if use_collective:
                nc.gpsimd.collective_compute(
                    "ReduceScatter",
                    mybir.AluOpType.add,
                    replica_groups=[[0, 1], [2, 3], [4, 5], [6, 7]],
                    ins=[pout.opt()],
                    outs=[rs_out.opt()],
                )
                nc.sync.dma_start(out_d[:], rs_out[:])
            else:
                nc.sync.dma_start(out_d[:], pout[:])

    nc.compile()
    return nc


def kernel(x, token_positions, W_q, W_k, W_v, W_o):
    from concourse.bass_utils import run_bass_kernel_spmd

    if "nc" not in _cache:
        _cache["nc"] = _build_program()
    nc = _cache["nc"]

    in_maps = _host_prep(x, token_positions, W_q, W_k, W_v, W_o)
    res = run_bass_kernel_spmd(nc, in_maps, list(range(N_CORES)))
    out = np.empty((B, S, D_MODEL), np.float32)
    for b in range(B):
        out[b, : S // 2] = res.results[2 * b]["out"]
        out[b, S // 2:] = res.results[2 * b + 1]["out"]
    return out


# revision 12
# speedup vs baseline: 2.0790x; 2.0790x over previous
"""Multi-head self-attention with RoPE — Trainium2 Bass/Tile kernel, 8 NeuronCores.

Sharding: batch x head tensor-parallel. Core pair (2b, 2b+1) handles batch b;
within a pair each core computes 8 of the 16 heads (W_q/W_k/W_v column-sharded,
W_o row-sharded), then a pairwise ReduceScatter sums the output-projection
partials and leaves each core with half of its batch's sequence rows.

Device layout notes:
 - All projections contract d_model on the partition dim; Q/K are produced
   transposed [d_k, seq] per head so attention scores come out transposed
   [k, q] ("S^T" layout): softmax reduction runs across partitions (GpSimd
   partition_all_reduce) and the AV matmul needs no transposes at all.
 - RoPE is applied via a host-side even/odd permutation of the W_q/W_k rows
   plus [cos;cos] and [sin;-sin] tables; the partition-half swap is done with
   two SBUF->SBUF DMAs.
 - No max-subtraction in softmax: scores here are bounded (|s| < ~10), exp is
   safe in f32/bf16. Causal masking adds -60 to masked diagonal-block entries
   before exp.
 - Matmuls run in bf16 with f32 PSUM accumulation; 1/sqrt(d_k) is folded into
   W_q on the host.
"""
import numpy as np
import ml_dtypes

D_MODEL = 2048
N_HEADS = 16
D_K = 128
B = 4
S = 2048
THETA = 10000.0
N_CORES = 8
HPC = N_HEADS // 2     # heads per core
HROWS = HPC * D_K      # 1024 = per-core projection width
NQT = S // 512         # 4 q-tiles of 512
NKC = S // 128         # 16 k-chunks of 128
NEG = -60.0
BF16 = ml_dtypes.bfloat16

_cache = {}


def _host_prep(x, token_positions, W_q, W_k, W_v, W_o):
    """Per-core input maps (sharding + layout prep, all host-side numpy)."""
    x = np.asarray(x, np.float32)
    W_q = np.asarray(W_q, np.float32)
    W_k = np.asarray(W_k, np.float32)
    W_v = np.asarray(W_v, np.float32)
    W_o = np.asarray(W_o, np.float32)
    pos = np.asarray(token_positions).astype(np.float32)

    half = D_K // 2
    inv_freq = (THETA ** (-(np.arange(half, dtype=np.float32) * 2.0) / D_K)).astype(np.float32)
    ang = pos[:, None] * inv_freq[None, :]          # [S, 64]
    cos = np.cos(ang).astype(np.float32).T          # [64, S]
    sin = np.sin(ang).astype(np.float32).T
    cos2 = np.concatenate([cos, cos], axis=0)                # [128, S] f32
    sin2 = np.concatenate([sin, -sin], axis=0)               # [128, S] f32

    perm = np.concatenate([np.arange(0, D_K, 2), np.arange(1, D_K, 2)])

    kl = np.arange(128)[:, None, None]
    dd = np.arange(4)[None, :, None]
    jj = np.arange(512)[None, None, :]
    masks = np.where(dd * 128 + kl <= jj, 0.0, NEG).astype(np.float32)  # [128,4,512]

    in_maps = []
    for c in range(N_CORES):
        b = c // 2
        hh = c % 2
        hsel = slice(hh * HROWS, (hh + 1) * HROWS)

        def permute_heads(Wrows):
            Wr = Wrows.reshape(HPC, D_K, D_MODEL)[:, perm, :]
            return Wr.reshape(HROWS, D_MODEL)

        wq = permute_heads(W_q[hsel]) / np.sqrt(np.float32(D_K))
        wk = permute_heads(W_k[hsel])
        wv = W_v[hsel]
        wo = W_o[:, hsel]                            # [2048, 1024]

        in_maps.append({
            "xT": np.ascontiguousarray(x[b].T).astype(BF16),     # [2048, 2048]
            "wqT": np.ascontiguousarray(wq.T).astype(BF16),      # [2048, 1024]
            "wkT": np.ascontiguousarray(wk.T).astype(BF16),      # [2048, 1024]
            "wvT": np.ascontiguousarray(wv.T).astype(BF16),      # [2048, 1024]
            "woT": np.ascontiguousarray(wo.T).astype(BF16),      # [1024, 2048]
            "cos2": cos2,
            "sin2": sin2,
            "masks": masks,
        })
    return in_maps


def _build_program(use_collective=True):
    import concourse.bass as bass
    import concourse.mybir as mybir
    import concourse.tile as tile
    from concourse import bacc, bass_isa

    f32 = mybir.dt.float32
    bf16 = mybir.dt.bfloat16
    EXP = mybir.ActivationFunctionType.Exp
    MUL = mybir.AluOpType.mult
    ADD = mybir.AluOpType.add

    nc = bacc.Bacc("TRN2", target_bir_lowering=False, debug=False,
                   num_devices=N_CORES)

    xT_d = nc.dram_tensor("xT", [D_MODEL, S], bf16, kind="ExternalInput")
    wqT_d = nc.dram_tensor("wqT", [D_MODEL, HROWS], bf16, kind="ExternalInput")
    wkT_d = nc.dram_tensor("wkT", [D_MODEL, HROWS], bf16, kind="ExternalInput")
    wvT_d = nc.dram_tensor("wvT", [D_MODEL, HROWS], bf16, kind="ExternalInput")
    woT_d = nc.dram_tensor("woT", [HROWS, D_MODEL], bf16, kind="ExternalInput")
    cos2_d = nc.dram_tensor("cos2", [128, S], f32, kind="ExternalInput")
    sin2_d = nc.dram_tensor("sin2", [128, S], f32, kind="ExternalInput")
    masks_d = nc.dram_tensor("masks", [128, 4, 512], f32, kind="ExternalInput")
    out_d = nc.dram_tensor("out", [S // 2 if use_collective else S, D_MODEL], f32,
                           kind="ExternalOutput")

    DM_CH = D_MODEL // 128  # 16 contraction chunks

    with tile.TileContext(nc) as tc:
        with (
            tc.tile_pool(name="const", bufs=1) as cpool,
            tc.tile_pool(name="big", bufs=1) as bigpool,
            tc.tile_pool(name="xs", bufs=2) as xpool,
            tc.tile_pool(name="w", bufs=2) as wpool,
            tc.tile_pool(name="qt", bufs=2) as qpool,
            tc.tile_pool(name="tmp", bufs=2) as tpool,
            tc.tile_pool(name="den", bufs=1) as dpool,
            tc.tile_pool(name="p", bufs=2) as ppool,
            tc.tile_pool(name="osb", bufs=2) as opool,
            tc.tile_pool(name="psum", bufs=2, space="PSUM") as psum,
            tc.tile_pool(name="psumS", bufs=3, space="PSUM") as psumS,
            tc.tile_pool(name="psumO", bufs=1, space="PSUM") as psumO,
            tc.tile_pool(name="dram", bufs=1, space="DRAM") as dram,
        ):
            # ---- constants ----
            cos2 = cpool.tile([128, S], f32, tag="cos2")
            sin2 = cpool.tile([128, S], f32, tag="sin2")
            masks = cpool.tile([128, 4, 512], f32, tag="masks")
            nc.gpsimd.dma_start(cos2[:], cos2_d[:])
            nc.gpsimd.dma_start(sin2[:], sin2_d[:])
            nc.gpsimd.dma_start(masks[:], masks_d[:])
            ones = cpool.tile([128, 1], bf16, tag="ones")
            nc.gpsimd.memset(ones[:], 1.0)

            # ---- persistent phase-A outputs ----
            kTr = bigpool.tile([128, HPC, S], bf16, tag="kTr")      # [dk, h, keys]
            v_sb = bigpool.tile([128, NKC, HROWS], bf16, tag="v")   # [row, kc, hdim]

            # DRAM bounce buffers for the collective
            pout = dram.tile([S, D_MODEL], f32, tag="pout")
            rs_out = dram.tile([S // 2, D_MODEL], f32, tag="rs_out")

            def rope_epilogue(ps, out_ap, ns):
                """out = ps*cos2 + swap(ps*sin2), cast bf16. ps: [128,512] psum."""
                u = tpool.tile([128, 512], f32, tag="u")
                t = tpool.tile([128, 512], f32, tag="t")
                usw = tpool.tile([128, 512], f32, tag="usw")
                nc.vector.tensor_tensor(u[:], ps[:], sin2[:, ns], MUL)
                nc.vector.tensor_tensor(t[:], ps[:], cos2[:, ns], MUL)
                nc.sync.dma_start(usw[0:64, :], u[64:128, :])
                nc.sync.dma_start(usw[64:128, :], u[0:64, :])
                nc.vector.tensor_tensor(out_ap, t[:], usw[:], ADD)

            xT_r = xT_d[:].rearrange("(c p) s -> p c s", p=128)     # [128, 16, S]

            # ---- per q-tile: K/V/Q projections for this slice + attention + O ----
            for qt in range(NQT):
                qs = slice(qt * 512, (qt + 1) * 512)
                xs = xpool.tile([128, DM_CH, 512], bf16, tag="xs")
                nc.sync.dma_start(xs[:], xT_r[:, :, qs])

                # K projection for key rows of this slice (transposed + RoPE)
                for m in range(HPC):
                    wt = wpool.tile([128, DM_CH, 128], bf16, tag="w")
                    nc.sync.dma_start(
                        wt[:],
                        wkT_d[:].rearrange("(c p) m -> p c m", p=128)[
                            :, :, m * 128:(m + 1) * 128],
                    )
                    ps = psum.tile([128, 512], f32, tag="proj")
                    for k in range(DM_CH):
                        nc.tensor.matmul(ps[:], wt[:, k, :], xs[:, k, :],
                                         start=(k == 0), stop=(k == DM_CH - 1))
                    rope_epilogue(ps, kTr[:, m, qs], qs)

                # V projection for key rows of this slice (natural layout)
                for nv in range(2):
                    nvs = slice(nv * 512, (nv + 1) * 512)
                    wv = wpool.tile([128, DM_CH, 512], bf16, tag="w")
                    nc.sync.dma_start(
                        wv[:],
                        wvT_d[:].rearrange("(c p) m -> p c m", p=128)[:, :, nvs],
                    )
                    for rc in range(4):
                        ps = psum.tile([128, 512], f32, tag="proj")
                        for k in range(DM_CH):
                            nc.tensor.matmul(
                                ps[:], xs[:, k, rc * 128:(rc + 1) * 128],
                                wv[:, k, :],
                                start=(k == 0), stop=(k == DM_CH - 1))
                        nc.vector.tensor_copy(v_sb[:, qt * 4 + rc, nvs], ps[:])

                # Q projection for this q-tile (transposed + RoPE)
                qTr = qpool.tile([128, HPC, 512], bf16, tag="qTr")
                for m in range(HPC):
                    wt = wpool.tile([128, DM_CH, 128], bf16, tag="w")
                    nc.sync.dma_start(
                        wt[:],
                        wqT_d[:].rearrange("(c p) m -> p c m", p=128)[
                            :, :, m * 128:(m + 1) * 128],
                    )
                    ps = psum.tile([128, 512], f32, tag="proj")
                    for k in range(DM_CH):
                        nc.tensor.matmul(ps[:], wt[:, k, :], xs[:, k, :],
                                         start=(k == 0), stop=(k == DM_CH - 1))
                    rope_epilogue(ps, qTr[:, m, :], qs)

                # attention for this q-tile (S^T layout, PE denominator)
                ctx_t = qpool.tile([128, HPC, 512], bf16, tag="ctx")
                nkc = 4 * (qt + 1)
                for h in range(HPC):
                    ctx_ps = psum.tile([128, 512], f32, tag="ctx")
                    den_ps = psumS.tile([1, 512], f32, tag="S")
                    for kc in range(nkc):
                        s_ps = psumS.tile([128, 512], f32, tag="S")
                        nc.tensor.matmul(
                            s_ps[:], kTr[:, h, kc * 128:(kc + 1) * 128],
                            qTr[:, h, :], start=True, stop=True)
                        d = kc - 4 * qt
                        if d >= 0:
                            nc.vector.tensor_tensor(s_ps[:], s_ps[:],
                                                    masks[:, d, :], ADD)
                        p_sb = ppool.tile([128, 512], bf16, tag="p")
                        nc.scalar.activation(p_sb[:], s_ps[:], EXP)
                        nc.tensor.matmul(
                            den_ps[:], ones[:], p_sb[:],
                            start=(kc == 0), stop=(kc == nkc - 1))
                        nc.tensor.matmul(
                            ctx_ps[:], v_sb[:, kc, h * 128:(h + 1) * 128],
                            p_sb[:], start=(kc == 0), stop=(kc == nkc - 1))
                    dsb = dpool.tile([1, 512], f32, tag="dsb")
                    nc.scalar.copy(dsb[:], den_ps[:])
                    rcpb = dpool.tile([128, 512], f32, tag="rcpb")
                    nc.gpsimd.partition_broadcast(rcpb[:], dsb[:])
                    nc.vector.reciprocal_approx_fast(rcpb[:], rcpb[:])
                    nc.vector.tensor_tensor(ctx_t[:, h, :], ctx_ps[:], rcpb[:], MUL)

                # O projection for this q-tile's rows -> partial out in DRAM
                for nt in range(4):
                    nts = slice(nt * 512, (nt + 1) * 512)
                    wo = wpool.tile([128, HPC, 512], bf16, tag="w")
                    nc.sync.dma_start(
                        wo[:],
                        woT_d[:].rearrange("(c p) m -> p c m", p=128)[:, :, nts],
                    )
                    for rc in range(4):
                        o_ps = psumO.tile([128, 512], f32, tag="O")
                        for h in range(HPC):
                            nc.tensor.matmul(
                                o_ps[:], ctx_t[:, h, rc * 128:(rc + 1) * 128],
                                wo[:, h, :], start=(h == 0), stop=(h == HPC - 1))
                        osb = opool.tile([128, 512], f32, tag="osb")
                        nc.vector.tensor_copy(osb[:], o_ps[:])
                        r0 = qt * 512 + rc * 128
                        nc.gpsimd.dma_start(pout[r0:r0 + 128, nts], osb[:])

                # after qt=1: first half of rows is complete -> early reduce-scatter
                if use_collective and qt == 1:
                    nc.gpsimd.collective_compute(
                        "ReduceScatter",
                        mybir.AluOpType.add,
                        replica_groups=[[0, 1], [2, 3], [4, 5], [6, 7]],
                        ins=[pout[0:1024, :].opt()],
                        outs=[rs_out[0:512, :].opt()],
                    )
                    nc.sync.dma_start(out_d[0:512, :], rs_out[0:512, :])

            # ---- second-half reduce-scatter + output ----
            if use_collective:
                nc.gpsimd.collective_compute(
                    "ReduceScatter",
                    mybir.AluOpType.add,
                    replica_groups=[[0, 1], [2, 3], [4, 5], [6, 7]],
                    ins=[pout[1024:2048, :].opt()],
                    outs=[rs_out[512:1024, :].opt()],
                )
                nc.sync.dma_start(out_d[512:1024, :], rs_out[512:1024, :])
            else:
                nc.sync.dma_start(out_d[:], pout[:])

    nc.compile()
    return nc


def kernel(x, token_positions, W_q, W_k, W_v, W_o):
    from concourse.bass_utils import run_bass_kernel_spmd

    if "nc" not in _cache:
        _cache["nc"] = _build_program()
    nc = _cache["nc"]

    in_maps = _host_prep(x, token_positions, W_q, W_k, W_v, W_o)
    res = run_bass_kernel_spmd(nc, in_maps, list(range(N_CORES)))
    return assemble([res.results[c]["out"] for c in range(N_CORES)])


def assemble(outs):
    """Stitch per-core [1024, 2048] RS outputs into [B, S, D_MODEL].

    The two reduce-scatters each split their row-range between the pair:
    core 2b holds batch-b rows 0:512 and 1024:1536, core 2b+1 holds rows
    512:1024 and 1536:2048."""
    out = np.empty((B, S, D_MODEL), np.float32)
    for b in range(B):
        out[b, 0:512] = outs[2 * b][0:512]
        out[b, 512:1024] = outs[2 * b + 1][0:512]
        out[b, 1024:1536] = outs[2 * b][512:1024]
        out[b, 1536:2048] = outs[2 * b + 1][512:1024]
    return out


# revision 13
# speedup vs baseline: 2.4232x; 1.1655x over previous
"""Multi-head self-attention with RoPE — Trainium2 Bass/Tile kernel, 8 NeuronCores.

Sharding: batch x head tensor-parallel. Core pair (2b, 2b+1) handles batch b;
within a pair each core computes 8 of the 16 heads (W_q/W_k/W_v column-sharded,
W_o row-sharded), then a pairwise ReduceScatter sums the output-projection
partials and leaves each core with half of its batch's sequence rows.

Device layout notes:
 - All projections contract d_model on the partition dim; Q/K are produced
   transposed [d_k, seq] per head so attention scores come out transposed
   [k, q] ("S^T" layout): softmax reduction runs across partitions (GpSimd
   partition_all_reduce) and the AV matmul needs no transposes at all.
 - RoPE is applied via a host-side even/odd permutation of the W_q/W_k rows
   plus [cos;cos] and [sin;-sin] tables; the partition-half swap is done with
   two SBUF->SBUF DMAs.
 - No max-subtraction in softmax: scores here are bounded (|s| < ~10), exp is
   safe in f32/bf16. Causal masking adds -60 to masked diagonal-block entries
   before exp.
 - Matmuls run in bf16 with f32 PSUM accumulation; 1/sqrt(d_k) is folded into
   W_q on the host.
"""
import numpy as np
import ml_dtypes

D_MODEL = 2048
N_HEADS = 16
D_K = 128
B = 4
S = 2048
THETA = 10000.0
N_CORES = 8
HPC = N_HEADS // 2     # heads per core
HROWS = HPC * D_K      # 1024 = per-core projection width
NQT = S // 512         # 4 q-tiles of 512
NKC = S // 128         # 16 k-chunks of 128
NEG = -60.0
BF16 = ml_dtypes.bfloat16

_cache = {}


def _host_prep(x, token_positions, W_q, W_k, W_v, W_o):
    """Per-core input maps (sharding + layout prep, all host-side numpy)."""
    x = np.asarray(x, np.float32)
    W_q = np.asarray(W_q, np.float32)
    W_k = np.asarray(W_k, np.float32)
    W_v = np.asarray(W_v, np.float32)
    W_o = np.asarray(W_o, np.float32)
    pos = np.asarray(token_positions).astype(np.float32)

    half = D_K // 2
    inv_freq = (THETA ** (-(np.arange(half, dtype=np.float32) * 2.0) / D_K)).astype(np.float32)
    ang = pos[:, None] * inv_freq[None, :]          # [S, 64]
    cos = np.cos(ang).astype(np.float32).T          # [64, S]
    sin = np.sin(ang).astype(np.float32).T
    cos2 = np.concatenate([cos, cos], axis=0)                # [128, S] f32
    sin2 = np.concatenate([sin, -sin], axis=0)               # [128, S] f32

    perm = np.concatenate([np.arange(0, D_K, 2), np.arange(1, D_K, 2)])

    kl = np.arange(128)[:, None, None]
    dd = np.arange(4)[None, :, None]
    jj = np.arange(512)[None, None, :]
    masks = np.where(dd * 128 + kl <= jj, 0.0, NEG).astype(np.float32)  # [128,4,512]

    in_maps = []
    for c in range(N_CORES):
        b = c // 2
        hh = c % 2
        hsel = slice(hh * HROWS, (hh + 1) * HROWS)

        def permute_heads(Wrows):
            Wr = Wrows.reshape(HPC, D_K, D_MODEL)[:, perm, :]
            return Wr.reshape(HROWS, D_MODEL)

        wq = permute_heads(W_q[hsel]) / np.sqrt(np.float32(D_K))
        wk = permute_heads(W_k[hsel])
        wv = W_v[hsel]
        wo = W_o[:, hsel]                            # [2048, 1024]

        in_maps.append({
            "xT": np.ascontiguousarray(x[b].T).astype(BF16),     # [2048, 2048]
            "wqT": np.ascontiguousarray(wq.T).astype(BF16),      # [2048, 1024]
            "wkT": np.ascontiguousarray(wk.T).astype(BF16),      # [2048, 1024]
            "wvT": np.ascontiguousarray(wv.T).astype(BF16),      # [2048, 1024]
            "woT": np.ascontiguousarray(wo.T).astype(BF16),      # [1024, 2048]
            "cos2": cos2,
            "sin2": sin2,
            "masks": masks,
        })
    return in_maps


def _build_program(use_collective=True):
    import concourse.bass as bass
    import concourse.mybir as mybir
    import concourse.tile as tile
    from concourse import bacc, bass_isa

    f32 = mybir.dt.float32
    bf16 = mybir.dt.bfloat16
    EXP = mybir.ActivationFunctionType.Exp
    MUL = mybir.AluOpType.mult
    ADD = mybir.AluOpType.add

    nc = bacc.Bacc("TRN2", target_bir_lowering=False, debug=False,
                   num_devices=N_CORES)

    xT_d = nc.dram_tensor("xT", [D_MODEL, S], bf16, kind="ExternalInput")
    wqT_d = nc.dram_tensor("wqT", [D_MODEL, HROWS], bf16, kind="ExternalInput")
    wkT_d = nc.dram_tensor("wkT", [D_MODEL, HROWS], bf16, kind="ExternalInput")
    wvT_d = nc.dram_tensor("wvT", [D_MODEL, HROWS], bf16, kind="ExternalInput")
    woT_d = nc.dram_tensor("woT", [HROWS, D_MODEL], bf16, kind="ExternalInput")
    cos2_d = nc.dram_tensor("cos2", [128, S], f32, kind="ExternalInput")
    sin2_d = nc.dram_tensor("sin2", [128, S], f32, kind="ExternalInput")
    masks_d = nc.dram_tensor("masks", [128, 4, 512], f32, kind="ExternalInput")
    out_d = nc.dram_tensor("out", [S // 2 if use_collective else S, D_MODEL],
                           bf16 if use_collective else f32, kind="ExternalOutput")

    DM_CH = D_MODEL // 128  # 16 contraction chunks

    with tile.TileContext(nc) as tc:
        with (
            tc.tile_pool(name="const", bufs=1) as cpool,
            tc.tile_pool(name="big", bufs=1) as bigpool,
            tc.tile_pool(name="xs", bufs=2) as xpool,
            tc.tile_pool(name="w", bufs=2) as wpool,
            tc.tile_pool(name="qt", bufs=2) as qpool,
            tc.tile_pool(name="tmp", bufs=2) as tpool,
            tc.tile_pool(name="den", bufs=1) as dpool,
            tc.tile_pool(name="p", bufs=2) as ppool,
            tc.tile_pool(name="osb", bufs=2) as opool,
            tc.tile_pool(name="psum", bufs=2, space="PSUM") as psum,
            tc.tile_pool(name="psumS", bufs=3, space="PSUM") as psumS,
            tc.tile_pool(name="psumO", bufs=1, space="PSUM") as psumO,
            tc.tile_pool(name="dram", bufs=1, space="DRAM") as dram,
        ):
            # ---- constants ----
            cos2 = cpool.tile([128, S], f32, tag="cos2")
            sin2 = cpool.tile([128, S], f32, tag="sin2")
            masks = cpool.tile([128, 4, 512], f32, tag="masks")
            nc.gpsimd.dma_start(cos2[:], cos2_d[:])
            nc.gpsimd.dma_start(sin2[:], sin2_d[:])
            nc.gpsimd.dma_start(masks[:], masks_d[:])
            ones = cpool.tile([128, 1], bf16, tag="ones")
            nc.gpsimd.memset(ones[:], 1.0)

            # ---- persistent phase-A outputs ----
            kTr = bigpool.tile([128, HPC, S], bf16, tag="kTr")      # [dk, h, keys]
            v_sb = bigpool.tile([128, NKC, HROWS], bf16, tag="v")   # [row, kc, hdim]

            # DRAM bounce buffers for the collective
            pout = dram.tile([S, D_MODEL], bf16 if use_collective else f32,
                             tag="pout")
            rs_out = dram.tile([S // 2, D_MODEL], bf16, tag="rs_out")

            def rope_epilogue(ps, out_ap, ns):
                """out = ps*cos2 + swap(ps*sin2), cast bf16. ps: [128,512] psum."""
                u = tpool.tile([128, 512], f32, tag="u")
                t = tpool.tile([128, 512], f32, tag="t")
                usw = tpool.tile([128, 512], f32, tag="usw")
                nc.vector.tensor_tensor(u[:], ps[:], sin2[:, ns], MUL)
                nc.vector.tensor_tensor(t[:], ps[:], cos2[:, ns], MUL)
                nc.scalar.dma_start(usw[0:64, :], u[64:128, :])
                nc.scalar.dma_start(usw[64:128, :], u[0:64, :])
                nc.vector.tensor_tensor(out_ap, t[:], usw[:], ADD)

            xT_r = xT_d[:].rearrange("(c p) s -> p c s", p=128)     # [128, 16, S]

            # ---- per q-tile: K/V/Q projections for this slice + attention + O ----
            for qt in range(NQT):
                qs = slice(qt * 512, (qt + 1) * 512)
                xs = xpool.tile([128, DM_CH, 512], bf16, tag="xs")
                nc.sync.dma_start(xs[:], xT_r[:, :, qs])

                # K projection for key rows of this slice (transposed + RoPE)
                for m in range(HPC):
                    wt = wpool.tile([128, DM_CH, 128], bf16, tag="w")
                    nc.sync.dma_start(
                        wt[:],
                        wkT_d[:].rearrange("(c p) m -> p c m", p=128)[
                            :, :, m * 128:(m + 1) * 128],
                    )
                    ps = psum.tile([128, 512], f32, tag="proj")
                    for k in range(DM_CH):
                        nc.tensor.matmul(ps[:], wt[:, k, :], xs[:, k, :],
                                         start=(k == 0), stop=(k == DM_CH - 1))
                    rope_epilogue(ps, kTr[:, m, qs], qs)

                # V projection for key rows of this slice (natural layout)
                for nv in range(2):
                    nvs = slice(nv * 512, (nv + 1) * 512)
                    wv = wpool.tile([128, DM_CH, 512], bf16, tag="w")
                    nc.sync.dma_start(
                        wv[:],
                        wvT_d[:].rearrange("(c p) m -> p c m", p=128)[:, :, nvs],
                    )
                    for rc in range(4):
                        ps = psum.tile([128, 512], f32, tag="proj")
                        for k in range(DM_CH):
                            nc.tensor.matmul(
                                ps[:], xs[:, k, rc * 128:(rc + 1) * 128],
                                wv[:, k, :],
                                start=(k == 0), stop=(k == DM_CH - 1))
                        nc.vector.tensor_copy(v_sb[:, qt * 4 + rc, nvs], ps[:])

                # Q projection for this q-tile (transposed + RoPE)
                qTr = qpool.tile([128, HPC, 512], bf16, tag="qTr")
                for m in range(HPC):
                    wt = wpool.tile([128, DM_CH, 128], bf16, tag="w")
                    nc.sync.dma_start(
                        wt[:],
                        wqT_d[:].rearrange("(c p) m -> p c m", p=128)[
                            :, :, m * 128:(m + 1) * 128],
                    )
                    ps = psum.tile([128, 512], f32, tag="proj")
                    for k in range(DM_CH):
                        nc.tensor.matmul(ps[:], wt[:, k, :], xs[:, k, :],
                                         start=(k == 0), stop=(k == DM_CH - 1))
                    rope_epilogue(ps, qTr[:, m, :], qs)

                # attention for this q-tile (S^T layout, PE denominator)
                ctx_t = qpool.tile([128, HPC, 512], bf16, tag="ctx")
                nkc = 4 * (qt + 1)
                for h in range(HPC):
                    ctx_ps = psum.tile([128, 512], f32, tag="ctx")
                    den_ps = psumO.tile([1, 512], f32, tag="O")
                    for kc in range(nkc):
                        s_ps = psumS.tile([128, 512], f32, tag="S")
                        nc.tensor.matmul(
                            s_ps[:], kTr[:, h, kc * 128:(kc + 1) * 128],
                            qTr[:, h, :], start=True, stop=True)
                        d = kc - 4 * qt
                        if d >= 0:
                            nc.vector.tensor_tensor(s_ps[:], s_ps[:],
                                                    masks[:, d, :], ADD)
                        p_sb = ppool.tile([128, 512], bf16, tag="p")
                        nc.scalar.activation(p_sb[:], s_ps[:], EXP)
                        nc.tensor.matmul(
                            den_ps[:], ones[:], p_sb[:],
                            start=(kc == 0), stop=(kc == nkc - 1))
                        nc.tensor.matmul(
                            ctx_ps[:], v_sb[:, kc, h * 128:(h + 1) * 128],
                            p_sb[:], start=(kc == 0), stop=(kc == nkc - 1))
                    dsb = dpool.tile([1, 512], f32, tag="dsb")
                    nc.scalar.copy(dsb[:], den_ps[:])
                    rcpb = dpool.tile([128, 512], f32, tag="rcpb")
                    nc.gpsimd.partition_broadcast(rcpb[:], dsb[:])
                    nc.vector.reciprocal_approx_fast(rcpb[:], rcpb[:])
                    nc.vector.tensor_tensor(ctx_t[:, h, :], ctx_ps[:], rcpb[:], MUL)

                # O projection for this q-tile's rows -> partial out in DRAM
                for nt in range(4):
                    nts = slice(nt * 512, (nt + 1) * 512)
                    wo = wpool.tile([128, HPC, 512], bf16, tag="w")
                    nc.sync.dma_start(
                        wo[:],
                        woT_d[:].rearrange("(c p) m -> p c m", p=128)[:, :, nts],
                    )
                    for rc in range(4):
                        o_ps = psumO.tile([128, 512], f32, tag="O")
                        for h in range(HPC):
                            nc.tensor.matmul(
                                o_ps[:], ctx_t[:, h, rc * 128:(rc + 1) * 128],
                                wo[:, h, :], start=(h == 0), stop=(h == HPC - 1))
                        osb = opool.tile([128, 512],
                                         bf16 if use_collective else f32,
                                         tag="osb")
                        nc.vector.tensor_copy(osb[:], o_ps[:])
                        r0 = qt * 512 + rc * 128
                        nc.gpsimd.dma_start(pout[r0:r0 + 128, nts], osb[:])

            # ---- reduce-scatter + output ----
            if use_collective:
                nc.gpsimd.collective_compute(
                    "ReduceScatter",
                    mybir.AluOpType.add,
                    replica_groups=[[0, 1], [2, 3], [4, 5], [6, 7]],
                    ins=[pout.opt()],
                    outs=[rs_out.opt()],
                )
                nc.sync.dma_start(out_d[:], rs_out[:])
            else:
                nc.sync.dma_start(out_d[:], pout[:])

    nc.compile()
    return nc


def kernel(x, token_positions, W_q, W_k, W_v, W_o):
    from concourse.bass_utils import run_bass_kernel_spmd

    if "nc" not in _cache:
        _cache["nc"] = _build_program()
    nc = _cache["nc"]

    in_maps = _host_prep(x, token_positions, W_q, W_k, W_v, W_o)
    res = run_bass_kernel_spmd(nc, in_maps, list(range(N_CORES)))
    return assemble([res.results[c]["out"] for c in range(N_CORES)])


def assemble(outs):
    """Stitch per-core [1024, 2048] RS outputs into [B, S, D_MODEL].

    The reduce-scatter splits rows between the pair: core 2b holds batch-b
    rows 0:1024, core 2b+1 holds rows 1024:2048. Outputs arrive bf16."""
    out = np.empty((B, S, D_MODEL), np.float32)
    for b in range(B):
        out[b, : S // 2] = outs[2 * b].astype(np.float32)
        out[b, S // 2:] = outs[2 * b + 1].astype(np.float32)
    return out


# revision 14
# speedup vs baseline: 2.4652x; 1.0174x over previous
"""Multi-head self-attention with RoPE — Trainium2 Bass/Tile kernel, 8 NeuronCores.

Sharding: batch x head tensor-parallel. Core pair (2b, 2b+1) handles batch b;
within a pair each core computes 8 of the 16 heads (W_q/W_k/W_v column-sharded,
W_o row-sharded), then a pairwise ReduceScatter sums the output-projection
partials and leaves each core with half of its batch's sequence rows.

Device layout notes:
 - All projections contract d_model on the partition dim; Q/K are produced
   transposed [d_k, seq] per head so attention scores come out transposed
   [k, q] ("S^T" layout): softmax reduction runs across partitions (GpSimd
   partition_all_reduce) and the AV matmul needs no transposes at all.
 - RoPE is applied via a host-side even/odd permutation of the W_q/W_k rows
   plus [cos;cos] and [sin;-sin] tables; the partition-half swap is done with
   two SBUF->SBUF DMAs.
 - No max-subtraction in softmax: scores here are bounded (|s| < ~10), exp is
   safe in f32/bf16. Causal masking adds -60 to masked diagonal-block entries
   before exp.
 - Matmuls run in bf16 with f32 PSUM accumulation; 1/sqrt(d_k) is folded into
   W_q on the host.
"""
import numpy as np
import ml_dtypes

D_MODEL = 2048
N_HEADS = 16
D_K = 128
B = 4
S = 2048
THETA = 10000.0
N_CORES = 8
HPC = N_HEADS // 2     # heads per core
HROWS = HPC * D_K      # 1024 = per-core projection width
NQT = S // 512         # 4 q-tiles of 512
NKC = S // 128         # 16 k-chunks of 128
NEG = -60.0
BF16 = ml_dtypes.bfloat16

_cache = {}


def _host_prep(x, token_positions, W_q, W_k, W_v, W_o):
    """Per-core input maps (sharding + layout prep, all host-side numpy)."""
    x = np.asarray(x, np.float32)
    W_q = np.asarray(W_q, np.float32)
    W_k = np.asarray(W_k, np.float32)
    W_v = np.asarray(W_v, np.float32)
    W_o = np.asarray(W_o, np.float32)
    pos = np.asarray(token_positions).astype(np.float32)

    half = D_K // 2
    inv_freq = (THETA ** (-(np.arange(half, dtype=np.float32) * 2.0) / D_K)).astype(np.float32)
    ang = pos[:, None] * inv_freq[None, :]          # [S, 64]
    cos = np.cos(ang).astype(np.float32).T          # [64, S]
    sin = np.sin(ang).astype(np.float32).T
    cos2 = np.concatenate([cos, cos], axis=0)                # [128, S] f32
    sin2 = np.concatenate([sin, -sin], axis=0)               # [128, S] f32

    perm = np.concatenate([np.arange(0, D_K, 2), np.arange(1, D_K, 2)])

    kl = np.arange(128)[:, None, None]
    dd = np.arange(4)[None, :, None]
    jj = np.arange(512)[None, None, :]
    masks = np.where(dd * 128 + kl <= jj, 0.0, NEG).astype(np.float32)  # [128,4,512]

    in_maps = []
    for c in range(N_CORES):
        b = c // 2
        hh = c % 2
        hsel = slice(hh * HROWS, (hh + 1) * HROWS)

        def permute_heads(Wrows):
            Wr = Wrows.reshape(HPC, D_K, D_MODEL)[:, perm, :]
            return Wr.reshape(HROWS, D_MODEL)

        wq = permute_heads(W_q[hsel]) / np.sqrt(np.float32(D_K))
        wk = permute_heads(W_k[hsel])
        wv = W_v[hsel]
        wo = W_o[:, hsel]                            # [2048, 1024]

        # DMA-optimal pre-tiling: [tile_idx, partition, chunk, cols] so each
        # (tile, partition) source run is contiguous (full-bandwidth DMA).
        xT = x[b].T.astype(BF16)                      # [2048 dm, 2048 rows]
        wqT, wkT, wvT = wq.T.astype(BF16), wk.T.astype(BF16), wv.T.astype(BF16)
        woT = wo.T.astype(BF16)                       # [1024, 2048]
        in_maps.append({
            "x_t": np.ascontiguousarray(
                xT.reshape(16, 128, 4, 512).transpose(2, 1, 0, 3)),   # [4,128,16,512]
            "wq_t": np.ascontiguousarray(
                wqT.reshape(16, 128, 8, 128).transpose(2, 1, 0, 3)),  # [8,128,16,128]
            "wk_t": np.ascontiguousarray(
                wkT.reshape(16, 128, 8, 128).transpose(2, 1, 0, 3)),  # [8,128,16,128]
            "wv_t": np.ascontiguousarray(
                wvT.reshape(16, 128, 2, 512).transpose(2, 1, 0, 3)),  # [2,128,16,512]
            "wo_t": np.ascontiguousarray(
                woT.reshape(8, 128, 4, 512).transpose(2, 1, 0, 3)),   # [4,128,8,512]
            "cos2": cos2,
            "sin2": sin2,
            "masks": masks,
        })
    return in_maps


def _build_program(use_collective=True):
    import concourse.bass as bass
    import concourse.mybir as mybir
    import concourse.tile as tile
    from concourse import bacc, bass_isa

    f32 = mybir.dt.float32
    bf16 = mybir.dt.bfloat16
    EXP = mybir.ActivationFunctionType.Exp
    MUL = mybir.AluOpType.mult
    ADD = mybir.AluOpType.add

    nc = bacc.Bacc("TRN2", target_bir_lowering=False, debug=False,
                   num_devices=N_CORES)

    x_td = nc.dram_tensor("x_t", [4, 128, 16, 512], bf16, kind="ExternalInput")
    wq_td = nc.dram_tensor("wq_t", [8, 128, 16, 128], bf16, kind="ExternalInput")
    wk_td = nc.dram_tensor("wk_t", [8, 128, 16, 128], bf16, kind="ExternalInput")
    wv_td = nc.dram_tensor("wv_t", [2, 128, 16, 512], bf16, kind="ExternalInput")
    wo_td = nc.dram_tensor("wo_t", [4, 128, 8, 512], bf16, kind="ExternalInput")
    cos2_d = nc.dram_tensor("cos2", [128, S], f32, kind="ExternalInput")
    sin2_d = nc.dram_tensor("sin2", [128, S], f32, kind="ExternalInput")
    masks_d = nc.dram_tensor("masks", [128, 4, 512], f32, kind="ExternalInput")
    out_d = nc.dram_tensor("out", [S // 2 if use_collective else S, D_MODEL],
                           bf16 if use_collective else f32, kind="ExternalOutput")

    DM_CH = D_MODEL // 128  # 16 contraction chunks

    with tile.TileContext(nc) as tc:
        with (
            tc.tile_pool(name="const", bufs=1) as cpool,
            tc.tile_pool(name="big", bufs=1) as bigpool,
            tc.tile_pool(name="xs", bufs=2) as xpool,
            tc.tile_pool(name="w", bufs=2) as wpool,
            tc.tile_pool(name="qt", bufs=2) as qpool,
            tc.tile_pool(name="tmp", bufs=2) as tpool,
            tc.tile_pool(name="den", bufs=1) as dpool,
            tc.tile_pool(name="p", bufs=2) as ppool,
            tc.tile_pool(name="osb", bufs=2) as opool,
            tc.tile_pool(name="psum", bufs=2, space="PSUM") as psum,
            tc.tile_pool(name="psumS", bufs=3, space="PSUM") as psumS,
            tc.tile_pool(name="psumO", bufs=1, space="PSUM") as psumO,
            tc.tile_pool(name="dram", bufs=1, space="DRAM") as dram,
        ):
            # ---- constants ----
            cos2 = cpool.tile([128, S], f32, tag="cos2")
            sin2 = cpool.tile([128, S], f32, tag="sin2")
            masks = cpool.tile([128, 4, 512], f32, tag="masks")
            nc.gpsimd.dma_start(cos2[:], cos2_d[:])
            nc.gpsimd.dma_start(sin2[:], sin2_d[:])
            nc.gpsimd.dma_start(masks[:], masks_d[:])
            ones = cpool.tile([128, 1], bf16, tag="ones")
            nc.gpsimd.memset(ones[:], 1.0)

            # ---- persistent phase-A outputs ----
            kTr = bigpool.tile([128, HPC, S], bf16, tag="kTr")      # [dk, h, keys]
            v_sb = bigpool.tile([128, NKC, HROWS], bf16, tag="v")   # [row, kc, hdim]

            # DRAM bounce buffers for the collective
            pout = dram.tile([S, D_MODEL], bf16 if use_collective else f32,
                             tag="pout")
            rs_out = dram.tile([S // 2, D_MODEL], bf16, tag="rs_out")

            def rope_epilogue(ps, out_ap, ns):
                """out = ps*cos2 + swap(ps*sin2), cast bf16. ps: [128,512] psum."""
                u = tpool.tile([128, 512], f32, tag="u")
                t = tpool.tile([128, 512], f32, tag="t")
                usw = tpool.tile([128, 512], f32, tag="usw")
                nc.vector.tensor_tensor(u[:], ps[:], sin2[:, ns], MUL)
                nc.vector.tensor_tensor(t[:], ps[:], cos2[:, ns], MUL)
                nc.scalar.dma_start(usw[0:64, :], u[64:128, :])
                nc.scalar.dma_start(usw[64:128, :], u[0:64, :])
                nc.vector.tensor_tensor(out_ap, t[:], usw[:], ADD)

            # ---- per q-tile: K/V/Q projections for this slice + attention + O ----
            for qt in range(NQT):
                qs = slice(qt * 512, (qt + 1) * 512)
                xs = xpool.tile([128, DM_CH, 512], bf16, tag="xs")
                nc.sync.dma_start(xs[:], x_td[qt])

                # K projection for key rows of this slice (transposed + RoPE)
                for m in range(HPC):
                    wt = wpool.tile([128, DM_CH, 128], bf16, tag="w")
                    nc.sync.dma_start(wt[:], wk_td[m])
                    ps = psum.tile([128, 512], f32, tag="proj")
                    for k in range(DM_CH):
                        nc.tensor.matmul(ps[:], wt[:, k, :], xs[:, k, :],
                                         start=(k == 0), stop=(k == DM_CH - 1))
                    rope_epilogue(ps, kTr[:, m, qs], qs)

                # V projection for key rows of this slice (natural layout)
                for nv in range(2):
                    nvs = slice(nv * 512, (nv + 1) * 512)
                    wv = wpool.tile([128, DM_CH, 512], bf16, tag="w")
                    nc.sync.dma_start(wv[:], wv_td[nv])
                    for rc in range(4):
                        ps = psum.tile([128, 512], f32, tag="proj")
                        for k in range(DM_CH):
                            nc.tensor.matmul(
                                ps[:], xs[:, k, rc * 128:(rc + 1) * 128],
                                wv[:, k, :],
                                start=(k == 0), stop=(k == DM_CH - 1))
                        nc.vector.tensor_copy(v_sb[:, qt * 4 + rc, nvs], ps[:])

                # Q projection for this q-tile (transposed + RoPE)
                qTr = qpool.tile([128, HPC, 512], bf16, tag="qTr")
                for m in range(HPC):
                    wt = wpool.tile([128, DM_CH, 128], bf16, tag="w")
                    nc.sync.dma_start(wt[:], wq_td[m])
                    ps = psum.tile([128, 512], f32, tag="proj")
                    for k in range(DM_CH):
                        nc.tensor.matmul(ps[:], wt[:, k, :], xs[:, k, :],
                                         start=(k == 0), stop=(k == DM_CH - 1))
                    rope_epilogue(ps, qTr[:, m, :], qs)

                # attention for this q-tile (S^T layout, PE denominator)
                ctx_t = qpool.tile([128, HPC, 512], bf16, tag="ctx")
                nkc = 4 * (qt + 1)
                for h in range(HPC):
                    ctx_ps = psum.tile([128, 512], f32, tag="ctx")
                    den_ps = psumO.tile([1, 512], f32, tag="O")
                    for kc in range(nkc):
                        s_ps = psumS.tile([128, 512], f32, tag="S")
                        nc.tensor.matmul(
                            s_ps[:], kTr[:, h, kc * 128:(kc + 1) * 128],
                            qTr[:, h, :], start=True, stop=True)
                        d = kc - 4 * qt
                        if d >= 0:
                            nc.vector.tensor_tensor(s_ps[:], s_ps[:],
                                                    masks[:, d, :], ADD)
                        p_sb = ppool.tile([128, 512], bf16, tag="p")
                        nc.scalar.activation(p_sb[:], s_ps[:], EXP)
                        nc.tensor.matmul(
                            den_ps[:], ones[:], p_sb[:],
                            start=(kc == 0), stop=(kc == nkc - 1))
                        nc.tensor.matmul(
                            ctx_ps[:], v_sb[:, kc, h * 128:(h + 1) * 128],
                            p_sb[:], start=(kc == 0), stop=(kc == nkc - 1))
                    dsb = dpool.tile([1, 512], f32, tag="dsb")
                    nc.scalar.copy(dsb[:], den_ps[:])
                    rcpb = dpool.tile([128, 512], f32, tag="rcpb")
                    nc.gpsimd.partition_broadcast(rcpb[:], dsb[:])
                    nc.vector.reciprocal_approx_fast(rcpb[:], rcpb[:])
                    nc.vector.tensor_tensor(ctx_t[:, h, :], ctx_ps[:], rcpb[:], MUL)

                # O projection for this q-tile's rows -> partial out in DRAM
                for nt in range(4):
                    nts = slice(nt * 512, (nt + 1) * 512)
                    wo = wpool.tile([128, HPC, 512], bf16, tag="w")
                    nc.sync.dma_start(wo[:], wo_td[nt])
                    for rc in range(4):
                        o_ps = psumO.tile([128, 512], f32, tag="O")
                        for h in range(HPC):
                            nc.tensor.matmul(
                                o_ps[:], ctx_t[:, h, rc * 128:(rc + 1) * 128],
                                wo[:, h, :], start=(h == 0), stop=(h == HPC - 1))
                        osb = opool.tile([128, 512],
                                         bf16 if use_collective else f32,
                                         tag="osb")
                        nc.vector.tensor_copy(osb[:], o_ps[:])
                        r0 = qt * 512 + rc * 128
                        nc.gpsimd.dma_start(pout[r0:r0 + 128, nts], osb[:])

            # ---- reduce-scatter + output ----
            if use_collective:
                nc.gpsimd.collective_compute(
                    "ReduceScatter",
                    mybir.AluOpType.add,
                    replica_groups=[[0, 1], [2, 3], [4, 5], [6, 7]],
                    ins=[pout.opt()],
                    outs=[rs_out.opt()],
                )
                nc.sync.dma_start(out_d[:], rs_out[:])
            else:
                nc.sync.dma_start(out_d[:], pout[:])

    nc.compile()
    return nc


def kernel(x, token_positions, W_q, W_k, W_v, W_o):
    from concourse.bass_utils import run_bass_kernel_spmd

    if "nc" not in _cache:
        _cache["nc"] = _build_program()
    nc = _cache["nc"]

    in_maps = _host_prep(x, token_positions, W_q, W_k, W_v, W_o)
    res = run_bass_kernel_spmd(nc, in_maps, list(range(N_CORES)))
    return assemble([res.results[c]["out"] for c in range(N_CORES)])


def assemble(outs):
    """Stitch per-core [1024, 2048] RS outputs into [B, S, D_MODEL].

    The reduce-scatter splits rows between the pair: core 2b holds batch-b
    rows 0:1024, core 2b+1 holds rows 1024:2048. Outputs arrive bf16."""
    out = np.empty((B, S, D_MODEL), np.float32)
    for b in range(B):
        out[b, : S // 2] = outs[2 * b].astype(np.float32)
        out[b, S // 2:] = outs[2 * b + 1].astype(np.float32)
    return out


# revision 16
# speedup vs baseline: 2.5626x; 1.0395x over previous
"""Multi-head self-attention with RoPE — Trainium2 Bass/Tile kernel, 8 NeuronCores.

Sharding: batch x head tensor-parallel. Core pair (2b, 2b+1) handles batch b;
within a pair each core computes 8 of the 16 heads (W_q/W_k/W_v column-sharded,
W_o row-sharded), then a pairwise ReduceScatter sums the output-projection
partials and leaves each core with half of its batch's sequence rows.

Device layout notes:
 - All projections contract d_model on the partition dim; Q/K are produced
   transposed [d_k, seq] per head so attention scores come out transposed
   [k, q] ("S^T" layout): softmax reduction runs across partitions (GpSimd
   partition_all_reduce) and the AV matmul needs no transposes at all.
 - RoPE is applied via a host-side even/odd permutation of the W_q/W_k rows
   plus [cos;cos] and [sin;-sin] tables; the partition-half swap is done with
   two SBUF->SBUF DMAs.
 - No max-subtraction in softmax: scores here are bounded (|s| < ~10), exp is
   safe in f32/bf16. Causal masking adds -60 to masked diagonal-block entries
   before exp.
 - Matmuls run in bf16 with f32 PSUM accumulation; 1/sqrt(d_k) is folded into
   W_q on the host.
"""
import numpy as np
import ml_dtypes

D_MODEL = 2048
N_HEADS = 16
D_K = 128
B = 4
S = 2048
THETA = 10000.0
N_CORES = 8
HPC = N_HEADS // 2     # heads per core
HROWS = HPC * D_K      # 1024 = per-core projection width
NQT = S // 512         # 4 q-tiles of 512
NKC = S // 128         # 16 k-chunks of 128
NEG = -60.0
BF16 = ml_dtypes.bfloat16

_cache = {}


def _host_prep(x, token_positions, W_q, W_k, W_v, W_o):
    """Per-core input maps (sharding + layout prep, all host-side numpy)."""
    x = np.asarray(x, np.float32)
    W_q = np.asarray(W_q, np.float32)
    W_k = np.asarray(W_k, np.float32)
    W_v = np.asarray(W_v, np.float32)
    W_o = np.asarray(W_o, np.float32)
    pos = np.asarray(token_positions).astype(np.float32)

    half = D_K // 2
    inv_freq = (THETA ** (-(np.arange(half, dtype=np.float32) * 2.0) / D_K)).astype(np.float32)
    ang = pos[:, None] * inv_freq[None, :]          # [S, 64]
    cos = np.cos(ang).astype(np.float32).T          # [64, S]
    sin = np.sin(ang).astype(np.float32).T
    cos2 = np.concatenate([cos, cos], axis=0)                # [128, S] f32
    sin2 = np.concatenate([sin, -sin], axis=0)               # [128, S] f32

    perm = np.concatenate([np.arange(0, D_K, 2), np.arange(1, D_K, 2)])

    kl = np.arange(128)[:, None, None]
    dd = np.arange(4)[None, :, None]
    jj = np.arange(512)[None, None, :]
    masks = np.where(dd * 128 + kl <= jj, 0.0, NEG).astype(np.float32)  # [128,4,512]

    in_maps = []
    for c in range(N_CORES):
        b = c // 2
        hh = c % 2
        hsel = slice(hh * HROWS, (hh + 1) * HROWS)

        def permute_heads(Wrows):
            Wr = Wrows.reshape(HPC, D_K, D_MODEL)[:, perm, :]
            return Wr.reshape(HROWS, D_MODEL)

        wq = permute_heads(W_q[hsel]) / np.sqrt(np.float32(D_K))
        wk = permute_heads(W_k[hsel])
        wv = W_v[hsel]
        wo = W_o[:, hsel]                            # [2048, 1024]

        # DMA-optimal pre-tiling: [tile_idx, partition, chunk, cols] so each
        # (tile, partition) source run is contiguous (full-bandwidth DMA).
        xT = x[b].T.astype(BF16)                      # [2048 dm, 2048 rows]
        wqT, wkT, wvT = wq.T.astype(BF16), wk.T.astype(BF16), wv.T.astype(BF16)
        woT = wo.T.astype(BF16)                       # [1024, 2048]
        in_maps.append({
            "x_t": np.ascontiguousarray(
                xT.reshape(16, 128, 4, 512).transpose(2, 1, 0, 3)),   # [4,128,16,512]
            "wq_t": np.ascontiguousarray(
                wqT.reshape(16, 128, 8, 128).transpose(2, 1, 0, 3)),  # [8,128,16,128]
            "wk_t": np.ascontiguousarray(
                wkT.reshape(16, 128, 8, 128).transpose(2, 1, 0, 3)),  # [8,128,16,128]
            "wv_t": np.ascontiguousarray(
                wvT.reshape(16, 128, 2, 512).transpose(2, 1, 0, 3)),  # [2,128,16,512]
            "wo_t": np.ascontiguousarray(
                woT.reshape(8, 128, 4, 512).transpose(2, 1, 0, 3)),   # [4,128,8,512]
            "cos2": cos2,
            "sin2": sin2,
            "masks": masks,
        })
    return in_maps


def _build_program(use_collective=True):
    import concourse.bass as bass
    import concourse.mybir as mybir
    import concourse.tile as tile
    from concourse import bacc, bass_isa

    f32 = mybir.dt.float32
    bf16 = mybir.dt.bfloat16
    EXP = mybir.ActivationFunctionType.Exp
    MUL = mybir.AluOpType.mult
    ADD = mybir.AluOpType.add

    nc = bacc.Bacc("TRN2", target_bir_lowering=False, debug=False,
                   num_devices=N_CORES)

    x_td = nc.dram_tensor("x_t", [4, 128, 16, 512], bf16, kind="ExternalInput")
    wq_td = nc.dram_tensor("wq_t", [8, 128, 16, 128], bf16, kind="ExternalInput")
    wk_td = nc.dram_tensor("wk_t", [8, 128, 16, 128], bf16, kind="ExternalInput")
    wv_td = nc.dram_tensor("wv_t", [2, 128, 16, 512], bf16, kind="ExternalInput")
    wo_td = nc.dram_tensor("wo_t", [4, 128, 8, 512], bf16, kind="ExternalInput")
    cos2_d = nc.dram_tensor("cos2", [128, S], f32, kind="ExternalInput")
    sin2_d = nc.dram_tensor("sin2", [128, S], f32, kind="ExternalInput")
    masks_d = nc.dram_tensor("masks", [128, 4, 512], f32, kind="ExternalInput")
    out_d = nc.dram_tensor("out", [S // 2 if use_collective else S, D_MODEL],
                           bf16 if use_collective else f32, kind="ExternalOutput")

    DM_CH = D_MODEL // 128  # 16 contraction chunks

    with tile.TileContext(nc) as tc:
        with (
            tc.tile_pool(name="const", bufs=1) as cpool,
            tc.tile_pool(name="big", bufs=1) as bigpool,
            tc.tile_pool(name="xs", bufs=2) as xpool,
            tc.tile_pool(name="w", bufs=2) as wpool,
            tc.tile_pool(name="qt", bufs=2) as qpool,
            tc.tile_pool(name="tmp", bufs=2) as tpool,
            tc.tile_pool(name="den", bufs=1) as dpool,
            tc.tile_pool(name="p", bufs=3) as ppool,
            tc.tile_pool(name="osb", bufs=2) as opool,
            tc.tile_pool(name="psum", bufs=2, space="PSUM") as psum,
            tc.tile_pool(name="psumS", bufs=3, space="PSUM") as psumS,
            tc.tile_pool(name="psumO", bufs=1, space="PSUM") as psumO,
            tc.tile_pool(name="dram", bufs=1, space="DRAM") as dram,
        ):
            # ---- constants ----
            cos2 = cpool.tile([128, S], f32, tag="cos2")
            sin2 = cpool.tile([128, S], f32, tag="sin2")
            masks = cpool.tile([128, 4, 512], f32, tag="masks")
            nc.scalar.dma_start(cos2[:], cos2_d[:])
            nc.sync.dma_start(sin2[:], sin2_d[:])
            nc.gpsimd.dma_start(masks[:], masks_d[:])
            ones = cpool.tile([128, 1], bf16, tag="ones")
            nc.gpsimd.memset(ones[:], 1.0)

            # ---- persistent phase-A outputs ----
            kTr = bigpool.tile([128, HPC, S], bf16, tag="kTr")      # [dk, h, keys]
            v_sb = bigpool.tile([128, NKC, HROWS], bf16, tag="v")   # [row, kc, hdim]

            # DRAM bounce buffers for the collective
            pout = dram.tile([S, D_MODEL], bf16 if use_collective else f32,
                             tag="pout")
            rs_out = dram.tile([S // 2, D_MODEL], bf16, tag="rs_out")

            def rope_epilogue(ps, out_ap, ns):
                """out = ps*cos2 + swap(ps*sin2), cast bf16. ps: [128,512] psum."""
                u = tpool.tile([128, 512], f32, tag="u")
                t = tpool.tile([128, 512], f32, tag="t")
                usw = tpool.tile([128, 512], f32, tag="usw")
                nc.vector.tensor_tensor(u[:], ps[:], sin2[:, ns], MUL)
                nc.vector.tensor_tensor(t[:], ps[:], cos2[:, ns], MUL)
                nc.scalar.dma_start(usw[0:64, :], u[64:128, :])
                nc.scalar.dma_start(usw[64:128, :], u[0:64, :])
                nc.vector.tensor_tensor(out_ap, t[:], usw[:], ADD)

            # ---- per q-tile: K/V/Q projections for this slice + attention + O ----
            for qt in range(NQT):
                qs = slice(qt * 512, (qt + 1) * 512)
                xs = xpool.tile([128, DM_CH, 512], bf16, tag="xs")
                nc.sync.dma_start(xs[:], x_td[qt])

                # Q projection for this q-tile (transposed + RoPE)
                qTr = qpool.tile([128, HPC, 512], bf16, tag="qTr")
                for m in range(HPC):
                    wt = wpool.tile([128, DM_CH, 128], bf16, tag="w")
                    nc.sync.dma_start(wt[:], wq_td[m])
                    ps = psum.tile([128, 512], f32, tag="proj")
                    for k in range(DM_CH):
                        nc.tensor.matmul(ps[:], wt[:, k, :], xs[:, k, :],
                                         start=(k == 0), stop=(k == DM_CH - 1))
                    rope_epilogue(ps, qTr[:, m, :], qs)

                # K projection for key rows of this slice (transposed + RoPE)
                for m in range(HPC):
                    wt = wpool.tile([128, DM_CH, 128], bf16, tag="w")
                    nc.sync.dma_start(wt[:], wk_td[m])
                    ps = psum.tile([128, 512], f32, tag="proj")
                    for k in range(DM_CH):
                        nc.tensor.matmul(ps[:], wt[:, k, :], xs[:, k, :],
                                         start=(k == 0), stop=(k == DM_CH - 1))
                    rope_epilogue(ps, kTr[:, m, qs], qs)

                # V projection for key rows of this slice (natural layout)
                for nv in range(2):
                    nvs = slice(nv * 512, (nv + 1) * 512)
                    wv = wpool.tile([128, DM_CH, 512], bf16, tag="w")
                    nc.sync.dma_start(wv[:], wv_td[nv])
                    for rc in range(4):
                        ps = psum.tile([128, 512], f32, tag="proj")
                        for k in range(DM_CH):
                            nc.tensor.matmul(
                                ps[:], xs[:, k, rc * 128:(rc + 1) * 128],
                                wv[:, k, :],
                                start=(k == 0), stop=(k == DM_CH - 1))
                        nc.vector.tensor_copy(v_sb[:, qt * 4 + rc, nvs], ps[:])

                # attention for this q-tile (S^T layout, PE denominator)
                ctx_t = qpool.tile([128, HPC, 512], bf16, tag="ctx")
                nkc = 4 * (qt + 1)
                for h in range(HPC):
                    ctx_ps = psum.tile([128, 512], f32, tag="ctx")
                    den_ps = psumO.tile([1, 512], f32, tag="O")
                    for kc in range(nkc):
                        s_ps = psumS.tile([128, 512], f32, tag="S")
                        nc.tensor.matmul(
                            s_ps[:], kTr[:, h, kc * 128:(kc + 1) * 128],
                            qTr[:, h, :], start=True, stop=True)
                        d = kc - 4 * qt
                        if d >= 0:
                            nc.vector.tensor_tensor(s_ps[:], s_ps[:],
                                                    masks[:, d, :], ADD)
                        p_sb = ppool.tile([128, 512], bf16, tag="p")
                        nc.scalar.activation(p_sb[:], s_ps[:], EXP)
                        nc.tensor.matmul(
                            den_ps[:], ones[:], p_sb[:],
                            start=(kc == 0), stop=(kc == nkc - 1))
                        nc.tensor.matmul(
                            ctx_ps[:], v_sb[:, kc, h * 128:(h + 1) * 128],
                            p_sb[:], start=(kc == 0), stop=(kc == nkc - 1))
                    dsb = dpool.tile([1, 512], f32, tag="dsb")
                    nc.scalar.copy(dsb[:], den_ps[:])
                    rcpb = dpool.tile([128, 512], f32, tag="rcpb")
                    nc.gpsimd.partition_broadcast(rcpb[:], dsb[:])
                    nc.vector.reciprocal_approx_fast(rcpb[:], rcpb[:])
                    nc.vector.tensor_tensor(ctx_t[:, h, :], ctx_ps[:], rcpb[:], MUL)

                # O projection for this q-tile's rows -> partial out in DRAM
                for nt in range(4):
                    nts = slice(nt * 512, (nt + 1) * 512)
                    wo = wpool.tile([128, HPC, 512], bf16, tag="w")
                    nc.sync.dma_start(wo[:], wo_td[nt])
                    for rc in range(4):
                        o_ps = psumO.tile([128, 512], f32, tag="O")
                        for h in range(HPC):
                            nc.tensor.matmul(
                                o_ps[:], ctx_t[:, h, rc * 128:(rc + 1) * 128],
                                wo[:, h, :], start=(h == 0), stop=(h == HPC - 1))
                        osb = opool.tile([128, 512],
                                         bf16 if use_collective else f32,
                                         tag="osb")
                        nc.vector.tensor_copy(osb[:], o_ps[:])
                        r0 = qt * 512 + rc * 128
                        nc.gpsimd.dma_start(pout[r0:r0 + 128, nts], osb[:])

            # ---- reduce-scatter + output ----
            if use_collective:
                nc.gpsimd.collective_compute(
                    "ReduceScatter",
                    mybir.AluOpType.add,
                    replica_groups=[[0, 1], [2, 3], [4, 5], [6, 7]],
                    ins=[pout.opt()],
                    outs=[rs_out.opt()],
                )
                nc.sync.dma_start(out_d[:], rs_out[:])
            else:
                nc.sync.dma_start(out_d[:], pout[:])

    nc.compile()
    return nc


def kernel(x, token_positions, W_q, W_k, W_v, W_o):
    from concourse.bass_utils import run_bass_kernel_spmd

    if "nc" not in _cache:
        _cache["nc"] = _build_program()
    nc = _cache["nc"]

    in_maps = _host_prep(x, token_positions, W_q, W_k, W_v, W_o)
    res = run_bass_kernel_spmd(nc, in_maps, list(range(N_CORES)))
    return assemble([res.results[c]["out"] for c in range(N_CORES)])


def assemble(outs):
    """Stitch per-core [1024, 2048] RS outputs into [B, S, D_MODEL].

    The reduce-scatter splits rows between the pair: core 2b holds batch-b
    rows 0:1024, core 2b+1 holds rows 1024:2048. Outputs arrive bf16."""
    out = np.empty((B, S, D_MODEL), np.float32)
    for b in range(B):
        out[b, : S // 2] = outs[2 * b].astype(np.float32)
        out[b, S // 2:] = outs[2 * b + 1].astype(np.float32)
    return out
